# revision 6
# baseline (speedup 1.0000x reference)
"""Trainium2 Bass kernel for BlockFFTDirectPrior.

Computes out = irfft(einsum('bjn,ijn->bin', rfft(x_blocks), conj(W)))
reshaped to [B, 4096], for x [4096, 4096] f32, W [16, 16, 129] complex
(block size 256).

Strategy: data-parallel over the batch axis across 8 NeuronCores (512 rows
each); W-derived constants replicated. Everything on-device is bf16
(operands) with fp32 PSUM accumulation; the 2e-2 harness gate leaves ~4x
margin over the ~5e-3 bf16 error. The host pre-transposes each core's x
shard into the [t, j, b] layout the DFT matmuls consume, so the kernel has
no on-device transpose stage at all. Three PE stages per core:

  F: real DFT as matmul (contract t, K=2x128 chunks)   -> X  [r, j, b]
       swizzled row r = f*16+g holds frequency n = 8g+f; the "sin" half's
       row 0 holds the (real) Nyquist bin.
  E: per-frequency 16x16 complex mixing as 8-frequency block-diagonal
     matmuls (K = (f,j) = 128)                         -> Y [(f,i), g, b]
  I: real inverse DFT with the data as the stationary operand, which
     restores the [b, m] orientation for free          -> out [b, i*256+m]

The two partition regroupings (F->E swaps j<->g, E->I swaps i<->g) are
plain affine SBUF->SBUF DMAs in bf16, split across the HWDGE (sync) and
SWDGE (gpsimd) rings. Input x and output land in HBM as bf16 (host
casts); together with bf16 intermediates this halves all DMA traffic
vs fp32.
"""

import os
import numpy as np
from contextlib import ExitStack

import ml_dtypes

import concourse.bass as bass
import concourse.tile as tile
from concourse import bacc, mybir
from concourse.bass_utils import run_bass_kernel_spmd

NCORES = 8
B_FULL, D_IN, D_OUT, BS = 4096, 4096, 4096, 256
BC = B_FULL // NCORES          # 512 batch rows per core
KIN = KOUT = 16
NG = 16                        # groups of 8 frequencies covering n=0..127
F32 = mybir.dt.float32
BF16 = mybir.dt.bfloat16
NPBF16 = ml_dtypes.bfloat16

_CACHE = {}
LAST_RESULTS = None            # BassKernelResults of the most recent run


# DFT/IDFT row swizzle: row r = f*16+g holds frequency n = 8g+f. This makes
# both partition regroups plain affine DMAs (partition dim outermost, step 1).
PERM = np.array([8 * (r % 16) + r // 16 for r in range(128)])


def _build_consts(W_real, W_imag):
    """Constant matrices in the exact SBUF layouts the kernel reads (bf16)."""
    t = np.arange(BS)
    n0 = np.arange(128)
    ang = 2.0 * np.pi / BS

    CF0 = np.cos(ang * np.outer(t, n0))
    CF1 = np.empty((BS, 128))
    CF1[:, 0] = np.cos(np.pi * t)
    p = np.arange(1, 128)
    CF1[:, 1:] = -np.sin(ang * np.outer(t, p))
    CF0 = CF0[:, PERM]
    CF1 = CF1[:, PERM]
    # cfs[tl, which, tc, r] = CF_which[tc*128 + tl, r]
    cfs = np.stack(
        [CF0.reshape(2, 128, 128), CF1.reshape(2, 128, 128)], axis=1
    ).transpose(2, 1, 0, 3)                                  # [128, 2, 2, 128]
    cfs = np.ascontiguousarray(cfs).astype(NPBF16)

    # wpk[(f*16+j), g, c, (f*16+i)] = M_c[i, j, 8g+f];  M = (Wr, Wi, -Wi)
    wpk = np.zeros((128, NG, 3, 128), dtype=np.float32)
    jj = np.arange(KIN)[:, None, None]
    ii = np.arange(KOUT)[None, :, None]
    ff = np.arange(8)[None, None, :]
    for g in range(NG):
        for c, M in enumerate((W_real, W_imag, -W_imag)):
            wpk[ff * 16 + jj, g, c, ff * 16 + ii] = M[ii, jj, 8 * g + ff]
    wpk = wpk.astype(NPBF16)
    wnyq = np.ascontiguousarray(W_real[:, :, 128].T).astype(NPBF16)  # [j, i]

    m = np.arange(BS)
    D0 = np.empty((128, BS))
    D0[0] = 1.0 / BS
    nn = np.arange(1, 128)
    D0[1:] = (2.0 / BS) * np.cos(ang * np.outer(nn, m))
    D1 = np.empty((128, BS))
    D1[0] = ((-1.0) ** m) / BS
    D1[1:] = -(2.0 / BS) * np.sin(ang * np.outer(nn, m))
    dmat = np.stack([D0[PERM], D1[PERM]], axis=1).astype(NPBF16)  # [128, 2, 256]
    return {"cfs": cfs, "wpk": wpk, "wnyq": wnyq, "dmat": dmat}


def _build_program():
    nc = bacc.Bacc(
        "TRN2", target_bir_lowering=False, debug=False, num_devices=NCORES
    )
    # x pre-transposed on the host: x_d[tl, j, tc, b] = x[b, j*256+tc*128+tl]
    x_d = nc.dram_tensor("x", [128, KIN, 2, BC], BF16, kind="ExternalInput").ap()
    cfs_d = nc.dram_tensor("cfs", [128, 2, 2, 128], BF16, kind="ExternalInput").ap()
    wpk_d = nc.dram_tensor("wpk", [128, NG, 3, 128], BF16, kind="ExternalInput").ap()
    wnyq_d = nc.dram_tensor("wnyq", [KIN, KOUT], BF16, kind="ExternalInput").ap()
    dmat_d = nc.dram_tensor("dmat", [128, 2, 256], BF16, kind="ExternalInput").ap()
    out_d = nc.dram_tensor("out", [BC, D_OUT], BF16, kind="ExternalOutput").ap()

    cp_state = [0]

    with tile.TileContext(nc) as tc, ExitStack() as ctx:
        def copy(dst, src):
            # alternate PSUM->SBUF copies between DVE and ACT
            if cp_state[0] % 2 == 0:
                nc.vector.tensor_copy(dst, src)
            else:
                nc.scalar.copy(dst, src)
            cp_state[0] += 1

        consts = ctx.enter_context(tc.tile_pool(name="consts", bufs=1))
        big = ctx.enter_context(tc.tile_pool(name="big", bufs=1))
        stg = ctx.enter_context(tc.tile_pool(name="stg", bufs=2))
        ps = ctx.enter_context(tc.tile_pool(name="ps", bufs=6, space="PSUM"))
        psn = ctx.enter_context(tc.tile_pool(name="psn", bufs=2, space="PSUM"))

        cfs = consts.tile([128, 2, 2, 128], BF16)
        wpk = consts.tile([128, NG, 3, 128], BF16)
        wnyq = consts.tile([KIN, KOUT], BF16)
        dmat = consts.tile([128, 2, 256], BF16)
        gnyq = consts.tile([KIN, BC], BF16)

        # consts first (tiny) so the PE warmup + stage F can start early
        nc.sync.dma_start(cfs[:], cfs_d)
        nc.sync.dma_start(dmat[:], dmat_d)
        nc.scalar.dma_start(wpk[:], wpk_d)
        nc.scalar.dma_start(wnyq[:], wnyq_d)

        xt = big.tile([128, KIN, 2, BC], BF16)   # (tl, j, tc, b)
        xf = big.tile([128, 2, KIN, BC], BF16)   # (r, which, j, b)
        gg = big.tile([128, 2, NG, BC], BF16)    # ((f,j), which, g, b)
        yy = big.tile([128, 2, NG, BC], BF16)    # ((f,i), half, g, b)
        yh = big.tile([128, 2, KOUT, BC], BF16)  # ((f,g), half, i, b)

        # ---- load x (already transposed on host), j-ascending for pipelining
        for jj in range(8):
            nc.sync.dma_start(
                xt[:, 2 * jj:2 * jj + 2, :, :], x_d[:, 2 * jj:2 * jj + 2, :, :]
            )

        # ---- PE warmup: dummy matmuls to lift the HAM clock gate while x
        # streams in (results thrown away via psn pool rotation)
        for w in range(16):
            pw = psn.tile([128, 256], F32, tag="psn")
            nc.tensor.matmul(
                pw[:], cfs[:, 0, 0, :], cfs[:, 1, :, :],
                start=True, stop=True,
            )

        # ---- stage F: real DFT (bf16 matmuls, fp32 PSUM)
        for j in range(KIN):
            for which in range(2):
                pf = ps.tile([128, BC], F32, tag="ps")
                nc.tensor.matmul(
                    pf[:], cfs[:, which, 0, :], xt[:, j, 0, :],
                    start=True, stop=False,
                )
                nc.tensor.matmul(
                    pf[:], cfs[:, which, 1, :], xt[:, j, 1, :],
                    start=False, stop=True,
                )
                copy(xf[:, which, j, :], pf[:])

        # Nyquist-real row (r=0 of the sin half) for all j
        nc.scalar.dma_start(gnyq[:], xf[0:1, 1, :, :])

        # ---- regroup1: gg[(f,j), which, g, b] = xf[f*16+g, which, j, b]
        rg_engs = [nc.sync, nc.gpsimd, nc.scalar]
        for g in range(NG):
            for which in range(2):
                eng = rg_engs[(2 * g + which) % 3]
                eng.dma_start(out=gg[:, which, g, :], in_=xf[g::16, which, :, :])

        # ---- stage E: blockdiag complex mixing
        for g in range(NG):
            pyr = ps.tile([128, BC], F32, tag="ps")
            nc.tensor.matmul(pyr[:], wpk[:, g, 0, :], gg[:, 0, g, :],
                             start=True, stop=False)
            nc.tensor.matmul(pyr[:], wpk[:, g, 1, :], gg[:, 1, g, :],
                             start=False, stop=True)
            copy(yy[:, 0, g, :], pyr[:])
            pyi = ps.tile([128, BC], F32, tag="ps")
            nc.tensor.matmul(pyi[:], wpk[:, g, 0, :], gg[:, 1, g, :],
                             start=True, stop=False)
            nc.tensor.matmul(pyi[:], wpk[:, g, 2, :], gg[:, 0, g, :],
                             start=False, stop=True)
            copy(yy[:, 1, g, :], pyi[:])
            if g == 0:
                # Nyquist einsum lands in the (f=0,g=0) rows of the imag half
                # (the otherwise meaningless Zi[0] slots); regroup2 then
                # routes it to yh row 0 of half 1 = dmat's Nyquist IDFT row.
                pyn = psn.tile([KIN, BC], F32, tag="psn")
                nc.tensor.matmul(pyn[:], wnyq[:], gnyq[:], start=True, stop=True)
                copy(yy[0:KIN, 1, 0, :], pyn[:])

        # ---- regroup2: yh[(f,g), half, i, b] = yy[f*16+i, half, g, b]
        for i in range(KOUT):
            for half in range(2):
                eng = rg_engs[(2 * i + half) % 3]
                eng.dma_start(out=yh[:, half, i, :], in_=yy[i::16, half, :, :])

        # ---- stage I: inverse DFT, data as stationary operand -> [b, m].
        # Two i-values share one PSUM bank so each copy moves a full bank.
        for bs in range(4):
            ot = stg.tile([128, D_OUT], BF16, tag="stg")
            for i2 in range(KOUT // 2):
                po = ps.tile([128, 2, BS], F32, tag="ps")
                for q in range(2):
                    i = 2 * i2 + q
                    nc.tensor.matmul(
                        po[:, q, :], yh[:, 0, i, 128 * bs:128 * (bs + 1)],
                        dmat[:, 0, :], start=True, stop=False)
                    nc.tensor.matmul(
                        po[:, q, :], yh[:, 1, i, 128 * bs:128 * (bs + 1)],
                        dmat[:, 1, :], start=False, stop=True)
                copy(ot[:, 512 * i2:512 * (i2 + 1)], po[:])
            nc.sync.dma_start(out_d[128 * bs:128 * (bs + 1), :], ot[:])

    nc.compile()
    return nc


def _get_program():
    if "nc" not in _CACHE:
        _CACHE["nc"] = _build_program()
    return _CACHE["nc"]


def _install_ntff_hook():
    """Provide antenv.axon_hooks (absent in this image) so that
    run_bass_kernel_spmd(trace=True) can capture NTFF profiles through the
    axon client library."""
    import sys
    import types
    import ctypes
    import contextlib

    if "antenv.axon_hooks" in sys.modules:
        return
    try:
        lib = ctypes.CDLL("/opt/axon/libaxon_pjrt.so")
    except OSError:
        return
    if not hasattr(lib, "axon_start_nrt_profile"):
        return
    lib.axon_start_nrt_profile.argtypes = [
        ctypes.POINTER(ctypes.c_int64),
        ctypes.c_size_t,
    ]
    lib.axon_start_nrt_profile.restype = ctypes.c_int64
    lib.axon_stop_nrt_profile.argtypes = [ctypes.c_char_p]
    lib.axon_stop_nrt_profile.restype = ctypes.c_int64

    @contextlib.contextmanager
    def _hook(output_dir, device_ids):
        import jax

        jax.devices()
        if device_ids:
            ids = (ctypes.c_int64 * len(device_ids))(*device_ids)
            rc = lib.axon_start_nrt_profile(ids, len(device_ids))
        else:
            rc = lib.axon_start_nrt_profile(None, 0)
        if rc != 0:
            raise RuntimeError(f"axon_start_nrt_profile rc={rc}")
        try:
            yield
        finally:
            n = lib.axon_stop_nrt_profile(str(output_dir).encode())
            print(f"ntff profile: {n} file(s) -> {output_dir}")

    mod = types.ModuleType("antenv.axon_hooks")
    state = {"hook": _hook}
    mod.get_axon_ntff_profile_hook = lambda: state["hook"]
    mod.set_axon_ntff_profile_hook = lambda h: state.update(hook=h)
    sys.modules["antenv.axon_hooks"] = mod
    import antenv

    antenv.axon_hooks = mod


def kernel(x, W_real, W_imag, block_size, out_features):
    global LAST_RESULTS
    x = np.asarray(x, dtype=np.float32)
    Wr = np.asarray(W_real, dtype=np.float32)
    Wi = np.asarray(W_imag, dtype=np.float32)
    assert int(block_size) == BS and int(out_features) == D_OUT
    assert x.shape == (B_FULL, D_IN) and Wr.shape == (KOUT, KIN, 129)

    nc = _get_program()
    consts = _build_consts(Wr, Wi)
    core_ids = list(range(NCORES))
    # host-side: cast to bf16 and transpose each core's shard into
    # xt[tl, j, tc, b] = x[b, j*256 + tc*128 + tl]
    xb = x.astype(NPBF16).reshape(NCORES, BC, KIN, 2, 128)
    in_maps = [
        {"x": np.ascontiguousarray(xb[c].transpose(3, 1, 2, 0)), **consts}
        for c in core_ids
    ]
    trace = bool(int(os.environ.get("KERNEL_TRACE", "0")))
    if trace:
        _install_ntff_hook()
    res = run_bass_kernel_spmd(nc, in_maps, core_ids, trace=trace)
    LAST_RESULTS = res
    out = np.concatenate([res.results[c]["out"] for c in core_ids], axis=0)
    return np.ascontiguousarray(out.astype(np.float32))


# revision 8
# speedup vs baseline: 1.0330x; 1.0330x over previous
"""Trainium2 Bass kernel for BlockFFTDirectPrior.

Computes out = irfft(einsum('bjn,ijn->bin', rfft(x_blocks), conj(W)))
reshaped to [B, 4096], for x [4096, 4096] f32, W [16, 16, 129] complex
(block size 256).

Strategy: data-parallel over the batch axis across 8 NeuronCores (512 rows
each); W-derived constants replicated. Everything on-device is bf16
(operands) with fp32 PSUM accumulation; the 2e-2 harness gate leaves ~4x
margin over the ~5e-3 bf16 error. The host pre-transposes each core's x
shard into the [t, j, b] layout the DFT matmuls consume, so the kernel has
no on-device transpose stage at all. Three PE stages per core:

  F: real DFT as matmul (contract t, K=2x128 chunks)   -> X  [r, j, b]
       swizzled row r = f*16+g holds frequency n = 8g+f; the "sin" half's
       row 0 holds the (real) Nyquist bin.
  E: per-frequency 16x16 complex mixing as 8-frequency block-diagonal
     matmuls (K = (f,j) = 128)                         -> Y [(f,i), g, b]
  I: real inverse DFT with the data as the stationary operand, which
     restores the [b, m] orientation for free          -> out [b, i*256+m]

The two partition regroupings (F->E swaps j<->g, E->I swaps i<->g) are
plain affine SBUF->SBUF DMAs in bf16, split across the HWDGE (sync) and
SWDGE (gpsimd) rings. Input x and output land in HBM as bf16 (host
casts); together with bf16 intermediates this halves all DMA traffic
vs fp32.
"""

import os
import numpy as np
from contextlib import ExitStack

import ml_dtypes

import concourse.bass as bass
import concourse.tile as tile
from concourse import bacc, mybir
from concourse.bass_utils import run_bass_kernel_spmd

NCORES = 8
B_FULL, D_IN, D_OUT, BS = 4096, 4096, 4096, 256
BC = B_FULL // NCORES          # 512 batch rows per core
KIN = KOUT = 16
NG = 16                        # groups of 8 frequencies covering n=0..127
F32 = mybir.dt.float32
BF16 = mybir.dt.bfloat16
NPBF16 = ml_dtypes.bfloat16

_CACHE = {}
LAST_RESULTS = None            # BassKernelResults of the most recent run


# DFT/IDFT row swizzle: row r = f*16+g holds frequency n = 8g+f. This makes
# both partition regroups plain affine DMAs (partition dim outermost, step 1).
PERM = np.array([8 * (r % 16) + r // 16 for r in range(128)])


def _build_consts(W_real, W_imag):
    """Constant matrices in the exact SBUF layouts the kernel reads (bf16)."""
    t = np.arange(BS)
    n0 = np.arange(128)
    ang = 2.0 * np.pi / BS

    CF0 = np.cos(ang * np.outer(t, n0))
    CF1 = np.empty((BS, 128))
    CF1[:, 0] = np.cos(np.pi * t)
    p = np.arange(1, 128)
    CF1[:, 1:] = -np.sin(ang * np.outer(t, p))
    CF0 = CF0[:, PERM]
    CF1 = CF1[:, PERM]
    # cfs[tl, which, tc, r] = CF_which[tc*128 + tl, r]
    cfs = np.stack(
        [CF0.reshape(2, 128, 128), CF1.reshape(2, 128, 128)], axis=1
    ).transpose(2, 1, 0, 3)                                  # [128, 2, 2, 128]
    cfs = np.ascontiguousarray(cfs).astype(NPBF16)

    # wpk[(f*16+j), g, c, (f*16+i)] = M_c[i, j, 8g+f];  M = (Wr, Wi, -Wi)
    wpk = np.zeros((128, NG, 3, 128), dtype=np.float32)
    jj = np.arange(KIN)[:, None, None]
    ii = np.arange(KOUT)[None, :, None]
    ff = np.arange(8)[None, None, :]
    for g in range(NG):
        for c, M in enumerate((W_real, W_imag, -W_imag)):
            wpk[ff * 16 + jj, g, c, ff * 16 + ii] = M[ii, jj, 8 * g + ff]
    wpk = wpk.astype(NPBF16)
    wnyq = np.ascontiguousarray(W_real[:, :, 128].T).astype(NPBF16)  # [j, i]

    m = np.arange(BS)
    D0 = np.empty((128, BS))
    D0[0] = 1.0 / BS
    nn = np.arange(1, 128)
    D0[1:] = (2.0 / BS) * np.cos(ang * np.outer(nn, m))
    D1 = np.empty((128, BS))
    D1[0] = ((-1.0) ** m) / BS
    D1[1:] = -(2.0 / BS) * np.sin(ang * np.outer(nn, m))
    dmat = np.stack([D0[PERM], D1[PERM]], axis=1).astype(NPBF16)  # [128, 2, 256]
    return {"cfs": cfs, "wpk": wpk, "wnyq": wnyq, "dmat": dmat}


def _build_program():
    nc = bacc.Bacc(
        "TRN2", target_bir_lowering=False, debug=False, num_devices=NCORES
    )
    # x pre-transposed on the host: x_d[tl, j, tc, b] = x[b, j*256+tc*128+tl]
    x_d = nc.dram_tensor("x", [128, KIN, 2, BC], BF16, kind="ExternalInput").ap()
    cfs_d = nc.dram_tensor("cfs", [128, 2, 2, 128], BF16, kind="ExternalInput").ap()
    wpk_d = nc.dram_tensor("wpk", [128, NG, 3, 128], BF16, kind="ExternalInput").ap()
    wnyq_d = nc.dram_tensor("wnyq", [KIN, KOUT], BF16, kind="ExternalInput").ap()
    dmat_d = nc.dram_tensor("dmat", [128, 2, 256], BF16, kind="ExternalInput").ap()
    out_d = nc.dram_tensor("out", [BC, D_OUT], BF16, kind="ExternalOutput").ap()

    cp_state = [0]

    with tile.TileContext(nc) as tc, ExitStack() as ctx:
        def copy(dst, src):
            # alternate PSUM->SBUF copies between DVE and ACT
            if cp_state[0] % 2 == 0:
                nc.vector.tensor_copy(dst, src)
            else:
                nc.scalar.copy(dst, src)
            cp_state[0] += 1

        consts = ctx.enter_context(tc.tile_pool(name="consts", bufs=1))
        big = ctx.enter_context(tc.tile_pool(name="big", bufs=1))
        stg = ctx.enter_context(tc.tile_pool(name="stg", bufs=2))
        ps = ctx.enter_context(tc.tile_pool(name="ps", bufs=6, space="PSUM"))
        psn = ctx.enter_context(tc.tile_pool(name="psn", bufs=2, space="PSUM"))

        cfs = consts.tile([128, 2, 2, 128], BF16)
        wpk = consts.tile([128, NG, 3, 128], BF16)
        wnyq = consts.tile([KIN, KOUT], BF16)
        dmat = consts.tile([128, 2, 256], BF16)
        gnyq = consts.tile([KIN, BC], BF16)

        # consts first (tiny) so the PE warmup + stage F can start early
        nc.sync.dma_start(cfs[:], cfs_d)
        nc.sync.dma_start(dmat[:], dmat_d)
        nc.scalar.dma_start(wpk[:], wpk_d)
        nc.scalar.dma_start(wnyq[:], wnyq_d)

        xt = big.tile([128, KIN, 2, BC], BF16)   # (tl, j, tc, b)
        xf = big.tile([128, 2, KIN, BC], BF16)   # (r, which, j, b)
        gg = big.tile([128, 2, NG, BC], BF16)    # ((f,j), which, g, b)
        yy = big.tile([128, 2, NG, BC], BF16)    # ((f,i), half, g, b)
        yh = big.tile([128, 2, KOUT, BC], BF16)  # ((f,g), half, i, b)

        # ---- load x (already transposed on host), j-ascending for pipelining
        for jj in range(8):
            nc.sync.dma_start(
                xt[:, 2 * jj:2 * jj + 2, :, :], x_d[:, 2 * jj:2 * jj + 2, :, :]
            )

        # ---- PE warmup: dummy matmuls to lift the HAM clock gate while x
        # streams in (results thrown away via psn pool rotation)
        for w in range(6):
            pw = psn.tile([128, 256], F32, tag="psn")
            nc.tensor.matmul(
                pw[:], cfs[:, 0, 0, :], cfs[:, 1, :, :],
                start=True, stop=True,
            )

        # ---- stage F: real DFT (bf16 matmuls, fp32 PSUM). The cos half runs
        # first (paced by the x load); its regroup then streams while the sin
        # half computes on the PE.
        for which in range(2):
            for j in range(KIN):
                pf = ps.tile([128, BC], F32, tag="ps")
                nc.tensor.matmul(
                    pf[:], cfs[:, which, 0, :], xt[:, j, 0, :],
                    start=True, stop=False,
                )
                nc.tensor.matmul(
                    pf[:], cfs[:, which, 1, :], xt[:, j, 1, :],
                    start=False, stop=True,
                )
                copy(xf[:, which, j, :], pf[:])
            # regroup1 for this half:
            # gg[(f,j), which, g, b] = xf[f*16+g, which, j, b]
            for g in range(NG):
                eng = nc.sync if g % 2 == 0 else nc.gpsimd
                eng.dma_start(out=gg[:, which, g, :], in_=xf[g::16, which, :, :])

        # Nyquist-real row (r=0 of the sin half) for all j
        nc.gpsimd.dma_start(gnyq[:], xf[0:1, 1, :, :])

        # ---- stage E: blockdiag complex mixing
        for g in range(NG):
            pyr = ps.tile([128, BC], F32, tag="ps")
            nc.tensor.matmul(pyr[:], wpk[:, g, 0, :], gg[:, 0, g, :],
                             start=True, stop=False)
            nc.tensor.matmul(pyr[:], wpk[:, g, 1, :], gg[:, 1, g, :],
                             start=False, stop=True)
            copy(yy[:, 0, g, :], pyr[:])
            pyi = ps.tile([128, BC], F32, tag="ps")
            nc.tensor.matmul(pyi[:], wpk[:, g, 0, :], gg[:, 1, g, :],
                             start=True, stop=False)
            nc.tensor.matmul(pyi[:], wpk[:, g, 2, :], gg[:, 0, g, :],
                             start=False, stop=True)
            copy(yy[:, 1, g, :], pyi[:])
            if g == 0:
                # Nyquist einsum lands in the (f=0,g=0) rows of the imag half
                # (the otherwise meaningless Zi[0] slots); regroup2 then
                # routes it to yh row 0 of half 1 = dmat's Nyquist IDFT row.
                pyn = psn.tile([KIN, BC], F32, tag="psn")
                nc.tensor.matmul(pyn[:], wnyq[:], gnyq[:], start=True, stop=True)
                copy(yy[0:KIN, 1, 0, :], pyn[:])

        # ---- regroup2: yh[(f,g), half, i, b] = yy[f*16+i, half, g, b]
        for i in range(KOUT):
            for half in range(2):
                eng = nc.sync if (2 * i + half) % 2 == 0 else nc.gpsimd
                eng.dma_start(out=yh[:, half, i, :], in_=yy[i::16, half, :, :])

        # ---- stage I: inverse DFT, data as stationary operand -> [b, m].
        # Two i-values share one PSUM bank so each copy moves a full bank.
        for bs in range(4):
            ot = stg.tile([128, D_OUT], BF16, tag="stg")
            for i2 in range(KOUT // 2):
                po = ps.tile([128, 2, BS], F32, tag="ps")
                for q in range(2):
                    i = 2 * i2 + q
                    nc.tensor.matmul(
                        po[:, q, :], yh[:, 0, i, 128 * bs:128 * (bs + 1)],
                        dmat[:, 0, :], start=True, stop=False)
                    nc.tensor.matmul(
                        po[:, q, :], yh[:, 1, i, 128 * bs:128 * (bs + 1)],
                        dmat[:, 1, :], start=False, stop=True)
                copy(ot[:, 512 * i2:512 * (i2 + 1)], po[:])
            nc.sync.dma_start(out_d[128 * bs:128 * (bs + 1), :], ot[:])

    nc.compile()
    return nc


def _get_program():
    if "nc" not in _CACHE:
        _CACHE["nc"] = _build_program()
    return _CACHE["nc"]


def _install_ntff_hook():
    """Provide antenv.axon_hooks (absent in this image) so that
    run_bass_kernel_spmd(trace=True) can capture NTFF profiles through the
    axon client library."""
    import sys
    import types
    import ctypes
    import contextlib

    if "antenv.axon_hooks" in sys.modules:
        return
    try:
        lib = ctypes.CDLL("/opt/axon/libaxon_pjrt.so")
    except OSError:
        return
    if not hasattr(lib, "axon_start_nrt_profile"):
        return
    lib.axon_start_nrt_profile.argtypes = [
        ctypes.POINTER(ctypes.c_int64),
        ctypes.c_size_t,
    ]
    lib.axon_start_nrt_profile.restype = ctypes.c_int64
    lib.axon_stop_nrt_profile.argtypes = [ctypes.c_char_p]
    lib.axon_stop_nrt_profile.restype = ctypes.c_int64

    @contextlib.contextmanager
    def _hook(output_dir, device_ids):
        import jax

        jax.devices()
        if device_ids:
            ids = (ctypes.c_int64 * len(device_ids))(*device_ids)
            rc = lib.axon_start_nrt_profile(ids, len(device_ids))
        else:
            rc = lib.axon_start_nrt_profile(None, 0)
        if rc != 0:
            raise RuntimeError(f"axon_start_nrt_profile rc={rc}")
        try:
            yield
        finally:
            n = lib.axon_stop_nrt_profile(str(output_dir).encode())
            print(f"ntff profile: {n} file(s) -> {output_dir}")

    mod = types.ModuleType("antenv.axon_hooks")
    state = {"hook": _hook}
    mod.get_axon_ntff_profile_hook = lambda: state["hook"]
    mod.set_axon_ntff_profile_hook = lambda h: state.update(hook=h)
    sys.modules["antenv.axon_hooks"] = mod
    import antenv

    antenv.axon_hooks = mod


def kernel(x, W_real, W_imag, block_size, out_features):
    global LAST_RESULTS
    x = np.asarray(x, dtype=np.float32)
    Wr = np.asarray(W_real, dtype=np.float32)
    Wi = np.asarray(W_imag, dtype=np.float32)
    assert int(block_size) == BS and int(out_features) == D_OUT
    assert x.shape == (B_FULL, D_IN) and Wr.shape == (KOUT, KIN, 129)

    nc = _get_program()
    consts = _build_consts(Wr, Wi)
    core_ids = list(range(NCORES))
    # host-side: cast to bf16 and transpose each core's shard into
    # xt[tl, j, tc, b] = x[b, j*256 + tc*128 + tl]
    xb = x.astype(NPBF16).reshape(NCORES, BC, KIN, 2, 128)
    in_maps = [
        {"x": np.ascontiguousarray(xb[c].transpose(3, 1, 2, 0)), **consts}
        for c in core_ids
    ]
    trace = bool(int(os.environ.get("KERNEL_TRACE", "0")))
    if trace:
        _install_ntff_hook()
    res = run_bass_kernel_spmd(nc, in_maps, core_ids, trace=trace)
    LAST_RESULTS = res
    out = np.concatenate([res.results[c]["out"] for c in core_ids], axis=0)
    return np.ascontiguousarray(out.astype(np.float32))
